# revision 1
# baseline (speedup 1.0000x reference)
"""Two-layer GraphSAGE (mean aggregation) on 8 Trainium2 NeuronCores.

Strategy (1D graph/data parallelism, edges partitioned by dst):
  - Core c owns dst nodes [c*NP, (c+1)*NP), NP = N/8.
  - Host sorts edges by (core, src-chunk, dst-block); src split into 4 chunks
    of <=32768 rows so dma_gather's int16 indices can address them.
  - Device: per 128-edge tile, dma_gather pulls x[src] rows (edge-major,
    [128 edges x 64 feat]); a 0/1 selection matrix S (built on DVE from
    per-edge dst-slot ids vs an iota row) turns segment-sum into a PE matmul
    accumulated in PSUM per dst-block; partial sums for a block are
    accumulated across chunks in a resident SBUF accumulator.
  - Block tail: scale by 1/deg, PE-transpose, two accumulated matmuls
    (aggr @ W_l.T + x_own @ W_r.T) + bias.
  - x1 shards are AllGather'd across the 8 cores between layers.
  - Weights (64x64) are replicated; the full output is the concat of shards.
"""
import sys

sys.path.insert(0, "/opt/trn_rl_repo")

import numpy as np

import concourse.bass as bass
import concourse.bacc as bacc
import concourse.mybir as mybir
import concourse.tile as tile
from concourse import bass_utils

P = 128
D = 64
M = 8          # cores
CH = 32768     # src chunk rows (int16-addressable)
GT = 8         # tiles per dma_gather instruction (HW caps num_idxs at 1024)
WB = 4         # dst blocks per batched tail DMA write

F32 = mybir.dt.float32
I16 = mybir.dt.int16

import os
_STAGE = int(os.environ.get("KERNEL_DEBUG_STAGE", "3"))  # 1=L1, 2=L1+AG, 3=full
_NO_XOWNT = bool(int(os.environ.get("K_NO_XOWNT", "0")))
_NO_INVD = bool(int(os.environ.get("K_NO_INVD", "0")))
_ONE_MM = bool(int(os.environ.get("K_ONE_MM", "0")))
_NO_BIAS = bool(int(os.environ.get("K_NO_BIAS", "0")))
_NO_TAIL = bool(int(os.environ.get("K_NO_TAIL", "0")))
_NO_GATHER = bool(int(os.environ.get("K_NO_GATHER", "0")))
_SINGLE_PACKET = bool(int(os.environ.get("K_SINGLE_PACKET", "1")))
_NQUEUES = int(os.environ.get("K_NQUEUES", "1"))
_NO_SBUILD = bool(int(os.environ.get("K_NO_SBUILD", "0")))
_NO_MM = bool(int(os.environ.get("K_NO_MM", "0")))

last_bass_results = None  # test.py reads exec_time_ns off this

_prog_cache = {}


def _build_schedule(src, dst, N, NP):
    """Host-side edge partitioning. Returns per-core device input arrays and
    the (shared across cores) tile schedule."""
    E = src.shape[0]
    NB = -(-NP // P)                      # dst blocks per core
    NQ = -(-N // CH)                      # src chunks

    deg = np.bincount(dst, minlength=N).astype(np.int64)

    core_e = dst // NP
    r = dst - core_e * NP
    blk_e = r // P
    slot_e = (r % P).astype(np.float32)
    q_e = src // CH
    loc_e = (src - q_e * CH).astype(np.int16)
    assert loc_e.min() >= 0

    key = (core_e * NQ + q_e) * NB + blk_e
    order = np.argsort(key, kind="stable")
    ks = key[order]

    cnt = np.bincount(key, minlength=M * NQ * NB).reshape(M, NQ, NB)
    nt = -(-cnt.max(axis=0) // P)          # [NQ, NB] tiles per (q, B)
    NT = int(nt.sum())
    NI = NT * P

    toff = np.zeros(NQ * NB, np.int64)
    toff[1:] = np.cumsum(nt.reshape(-1))[:-1]
    toff = toff.reshape(NQ, NB)

    # per-edge flat position inside its core's index array
    change = np.empty(E, bool)
    change[0] = True
    change[1:] = ks[1:] != ks[:-1]
    run_start = np.flatnonzero(change)
    run_id = np.cumsum(change) - 1
    rank = np.arange(E) - run_start[run_id]
    kq = (ks // NB) % NQ
    kB = ks % NB
    pos = toff[kq, kB] * P + rank
    c_e = ks // (NQ * NB)

    idxflat = np.zeros((M, NI), np.int16)
    slotflat = np.full((M, NI), -1.0, np.float32)
    idxflat[c_e, pos] = loc_e[order]
    slotflat[c_e, pos] = slot_e[order]

    # dma_gather wrap: index i -> [i%16, i//16], replicated over 8 groups
    idx_w = np.ascontiguousarray(
        np.tile(idxflat.reshape(M, NI // 16, 16).transpose(0, 2, 1), (1, 8, 1))
    )
    slot_w = np.ascontiguousarray(slotflat.reshape(M, NT, P).transpose(0, 2, 1))

    degp = np.ones((M, NB * P), np.float32)
    degp[:, :NP] = np.maximum(deg.reshape(M, NP), 1)
    invd_w = np.ascontiguousarray(
        (1.0 / degp).reshape(M, NB, P).transpose(0, 2, 1).astype(np.float32)
    )

    # schedule: per chunk, the global tile range; groups of <= GT tiles
    groups = []        # (q, g0, g1)
    tile_meta = []     # per global tile t: (block, first_of_run, last_of_run)
    for q in range(NQ):
        for B in range(NB):
            n = int(nt[q, B])
            for j in range(n):
                tile_meta.append((B, j == 0, j == n - 1))
        t0 = int(toff[q, 0])
        t_end = t0 + int(nt[q].sum())
        g = t0
        while g < t_end:
            g1 = min(g + GT, t_end)
            groups.append((q, g, g1))
            g = g1

    chunk_rows = [min(CH, N - q * CH) for q in range(NQ)]
    sched = {
        "N": N, "NP": NP, "NB": NB, "NQ": NQ, "NT": NT, "NI": NI,
        "groups": groups, "tile_meta": tile_meta, "chunk_rows": chunk_rows,
    }
    return sched, idx_w, slot_w, invd_w


def _build_program(sched):
    N, NP, NB, NQ, NT, NI = (
        sched["N"], sched["NP"], sched["NB"], sched["NQ"], sched["NT"], sched["NI"]
    )
    groups, tile_meta, chunk_rows = (
        sched["groups"], sched["tile_meta"], sched["chunk_rows"]
    )

    nc = bacc.Bacc(
        "TRN2", num_devices=M, num_swdge_queues=_NQUEUES,
        dynamic_dma_scratch_size=int(os.environ.get("K_SCRATCH", "65536")),
    )

    x_store = nc.dram_tensor("x_store", [N, D], F32, kind="ExternalInput")
    x_ownT = nc.dram_tensor("x_ownT", [D, NB * P], F32, kind="ExternalInput")
    idx16 = nc.dram_tensor("idx16", [P, NI // 16], I16, kind="ExternalInput")
    slots = nc.dram_tensor("slots", [P, NT], F32, kind="ExternalInput")
    invd = nc.dram_tensor("invd", [P, NB], F32, kind="ExternalInput")
    w1l = nc.dram_tensor("w1l", [D, D], F32, kind="ExternalInput")
    w1r = nc.dram_tensor("w1r", [D, D], F32, kind="ExternalInput")
    w2l = nc.dram_tensor("w2l", [D, D], F32, kind="ExternalInput")
    w2r = nc.dram_tensor("w2r", [D, D], F32, kind="ExternalInput")
    b1 = nc.dram_tensor("b1", [P, D], F32, kind="ExternalInput")
    b2 = nc.dram_tensor("b2", [P, D], F32, kind="ExternalInput")
    iota_in = nc.dram_tensor("iota", [P, P], F32, kind="ExternalInput")
    ident_in = nc.dram_tensor("ident", [P, P], F32, kind="ExternalInput")
    out_shard = nc.dram_tensor("out_shard", [NP, D], F32, kind="ExternalOutput")

    with tile.TileContext(nc) as tc:
        with (
            tc.tile_pool(name="const", bufs=1) as cpool,
            tc.tile_pool(name="res", bufs=1) as rpool,
            tc.tile_pool(name="gpool", bufs=4) as gpool,
            tc.tile_pool(name="spool", bufs=4) as spool,
            tc.tile_pool(name="wpool", bufs=3) as wpool,
            tc.tile_pool(name="paggr", bufs=3, space="PSUM") as paggr,
            tc.tile_pool(name="ptr", bufs=2, space="PSUM") as ptr,
            tc.tile_pool(name="pout", bufs=2, space="PSUM") as pout,
            tc.tile_pool(name="dram", bufs=1, space="DRAM") as dram,
        ):
            iota_sb = cpool.tile([P, P], F32)
            nc.sync.dma_start(iota_sb[:], iota_in[:])
            ident_sb = cpool.tile([P, P], F32)
            nc.sync.dma_start(ident_sb[:], ident_in[:])
            wl_sb, wr_sb, bias_sb = [], [], []
            for i, wsrc in enumerate((w1l, w2l)):
                t = cpool.tile([D, D], F32, tag=f"wl{i}")
                nc.sync.dma_start(t[:], wsrc[:])
                wl_sb.append(t)
            for i, wsrc in enumerate((w1r, w2r)):
                t = cpool.tile([D, D], F32, tag=f"wr{i}")
                nc.sync.dma_start(t[:], wsrc[:])
                wr_sb.append(t)
            for i, bsrc in enumerate((b1, b2)):
                t = cpool.tile([P, D], F32, tag=f"bias{i}")
                nc.sync.dma_start(t[:], bsrc[:])
                bias_sb.append(t)
            invd_sb = cpool.tile([P, NB], F32)
            nc.sync.dma_start(invd_sb[:], invd[:])
            xoT_sb = cpool.tile([D, NB * P], F32)
            nc.sync.dma_start(xoT_sb[:], x_ownT[:])
            # resident gather indices / dst-slot ids (one big contiguous load
            # each instead of per-group strided loads)
            idx_res = cpool.tile([P, NI // 16], I16)
            nc.sync.dma_start(idx_res[:], idx16[:])
            slot_res = cpool.tile([P, NT], F32)
            nc.sync.dma_start(slot_res[:], slots[:])

            x1_sb = rpool.tile([P, NB * D], F32, tag="x1_sb")
            x1_shard = dram.tile([NP, D], F32)
            x1_full = dram.tile([N, D], F32)

            nlayers = 1 if _STAGE < 3 else 2
            for layer in range(nlayers):
                acc = rpool.tile([P, NB * D], F32, tag="acc")
                nc.vector.memset(acc[:], 0.0)

                cur_pag = {}  # B -> open PSUM accumulation tile (may span groups)
                _gq = 0
                for (q, g0, g1) in groups:
                    ntg = g1 - g0
                    ni = ntg * P
                    base = q * CH
                    rows = chunk_rows[q]
                    if layer == 0:
                        src_ap = x_store[base : base + rows, :]
                    else:
                        src_ap = x1_full[base : base + rows, :]

                    g = gpool.tile([P, ntg * D], F32, tag="g")
                    if _NO_GATHER:
                        nc.vector.memset(g[:], 1.0)
                    else:
                        nc.gpsimd.dma_gather(
                            out_ap=g[:].rearrange("p (t d) -> p t d", t=ntg, d=D),
                            in_ap=src_ap,
                            idxs_ap=idx_res[:, g0 * 8 : g0 * 8 + ni // 16],
                            num_idxs=ni,
                            num_idxs_reg=ni,
                            elem_size=D,
                            elem_step=D,
                            single_packet=_SINGLE_PACKET,
                            queue_num=_gq % _NQUEUES,
                        )
                        _gq += 1

                    for t in range(g0, g1):
                        B, first, last = tile_meta[t]
                        if _NO_MM:
                            continue
                        S = spool.tile([P, P], F32, tag="S")
                        if _NO_SBUILD:
                            if t == g0:
                                nc.vector.memset(S[:], 0.0)
                        else:
                            nc.vector.tensor_scalar(
                                S[:], iota_sb[:], slot_res[:, t : t + 1],
                                None, mybir.AluOpType.is_equal,
                            )
                        if first:
                            cur_pag[B] = paggr.tile(
                                [P, D], F32, name="pag", tag="pag"
                            )
                        pag = cur_pag[B]
                        nc.tensor.matmul(
                            pag[:], lhsT=S[:],
                            rhs=g[:, (t - g0) * D : (t - g0 + 1) * D],
                            start=first, stop=last,
                        )
                        if last:
                            asl = acc[:, B * D : (B + 1) * D]
                            nc.vector.tensor_tensor(
                                out=asl, in0=asl, in1=pag[:],
                                op=mybir.AluOpType.add,
                            )

                for B in range(NB):
                    if _NO_TAIL:
                        rows = min(P, NP - B * P)
                        nc.sync.dma_start(
                            x1_shard[B * P : B * P + rows, :],
                            acc[:rows, B * D : (B + 1) * D],
                        )
                        if layer == 0:
                            nc.vector.tensor_copy(
                                x1_sb[:, B * D : (B + 1) * D],
                                acc[:, B * D : (B + 1) * D],
                            )
                        continue
                    rows = min(P, NP - B * P)
                    asl = acc[:, B * D : (B + 1) * D]
                    if not _NO_INVD:
                        nc.vector.tensor_scalar_mul(
                            asl, asl, invd_sb[:, B : B + 1]
                        )
                    pt = ptr.tile([D, P], F32, tag="ptr")
                    nc.tensor.transpose(pt[:], asl, ident_sb[:])
                    accT = wpool.tile([D, P], F32, tag="accT")
                    nc.vector.tensor_copy(accT[:], pt[:])
                    if _NO_XOWNT:
                        xT = accT[:]
                    elif layer == 0:
                        xT = xoT_sb[:, B * P : (B + 1) * P]
                    else:
                        pt2 = ptr.tile([D, P], F32, tag="ptr")
                        nc.tensor.transpose(
                            pt2[:], x1_sb[:, B * D : (B + 1) * D], ident_sb[:]
                        )
                        xTt = wpool.tile([D, P], F32, tag="xT")
                        nc.vector.tensor_copy(xTt[:], pt2[:])
                        xT = xTt[:]
                    po = pout.tile([P, D], F32)
                    nc.tensor.matmul(
                        po[:], lhsT=accT[:], rhs=wl_sb[layer][:],
                        start=True, stop=_ONE_MM,
                    )
                    if not _ONE_MM:
                        nc.tensor.matmul(
                            po[:], lhsT=xT, rhs=wr_sb[layer][:],
                            start=False, stop=True,
                        )
                    # L2 reuses acc slices as output staging (block B's acc is
                    # dead after its transpose)
                    dst_res = x1_sb if layer == 0 else acc
                    osl = dst_res[:, B * D : (B + 1) * D]
                    if _NO_BIAS:
                        nc.vector.tensor_copy(osl, po[:])
                    else:
                        nc.vector.tensor_tensor(
                            out=osl, in0=po[:], in1=bias_sb[layer][:],
                            op=mybir.AluOpType.add,
                        )
                    dst_dram = x1_shard if layer == 0 else out_shard
                    # batched write: flush every WB full blocks (or tail)
                    if B == NB - 1 or (B % WB == WB - 1 and (B + 1) * P <= NP):
                        b0 = (B // WB) * WB
                        nblk = B - b0 + 1
                        r0 = b0 * P
                        r1 = min(NP, (B + 1) * P)
                        if nblk > 1 and r1 == (B + 1) * P:
                            nc.sync.dma_start(
                                dst_dram[r0:r1, :].rearrange(
                                    "(j p) d -> p j d", p=P
                                ),
                                dst_res[:, b0 * D : (B + 1) * D].rearrange(
                                    "p (j d) -> p j d", d=D
                                ),
                            )
                        else:
                            for Bj in range(b0, B + 1):
                                rj0 = Bj * P
                                rj1 = min(NP, (Bj + 1) * P)
                                nc.sync.dma_start(
                                    dst_dram[rj0:rj1, :],
                                    dst_res[: rj1 - rj0, Bj * D : (Bj + 1) * D],
                                )

                if layer == 0 and _STAGE >= 2:
                    nc.gpsimd.collective_compute(
                        "AllGather",
                        mybir.AluOpType.bypass,
                        replica_groups=[list(range(M))],
                        ins=[x1_shard.opt()],
                        outs=[x1_full.opt()],
                    )
                if layer == 0 and _STAGE < 3:
                    # debug: emit x1 as the output so the program has a writer
                    for B in range(NB):
                        rows = min(P, NP - B * P)
                        nc.sync.dma_start(
                            out_shard[B * P : B * P + rows, :],
                            x1_sb[:rows, B * D : (B + 1) * D],
                        )

    nc.compile()
    return nc


def _prepare(x, edge_index, W1_l, b1_l, W1_r, W2_l, b2_l, W2_r):
    N, _D = x.shape
    assert _D == D and N % M == 0
    NP = N // M

    src = np.asarray(edge_index[0], dtype=np.int64)
    dst = np.asarray(edge_index[1], dtype=np.int64)

    sched, idx_w, slot_w, invd_w = _build_schedule(src, dst, N, NP)
    NB = sched["NB"]

    ck = (N, NP, sched["NT"], tuple(sched["groups"]),
          tuple(t for t in sched["tile_meta"]))
    import hashlib
    hk = hashlib.sha1(repr(ck).encode()).hexdigest()
    if hk not in _prog_cache:
        _prog_cache[hk] = _build_program(sched)
    nc = _prog_cache[hk]

    x = np.ascontiguousarray(np.asarray(x, np.float32))
    xoT = np.zeros((M, D, NB * P), np.float32)
    xr = x.reshape(M, NP, D)
    xoT[:, :, :NP] = xr.transpose(0, 2, 1)

    w1l_np = np.ascontiguousarray(np.asarray(W1_l, np.float32).T)
    w1r_np = np.ascontiguousarray(np.asarray(W1_r, np.float32).T)
    w2l_np = np.ascontiguousarray(np.asarray(W2_l, np.float32).T)
    w2r_np = np.ascontiguousarray(np.asarray(W2_r, np.float32).T)
    b1_np = np.ascontiguousarray(
        np.broadcast_to(np.asarray(b1_l, np.float32), (P, D))
    )
    b2_np = np.ascontiguousarray(
        np.broadcast_to(np.asarray(b2_l, np.float32), (P, D))
    )
    iota_np = np.ascontiguousarray(
        np.tile(np.arange(P, dtype=np.float32), (P, 1))
    )
    ident_np = np.eye(P, dtype=np.float32)

    in_maps = []
    for c in range(M):
        in_maps.append({
            "x_store": x,
            "x_ownT": np.ascontiguousarray(xoT[c]),
            "idx16": idx_w[c],
            "slots": slot_w[c],
            "invd": invd_w[c],
            "w1l": w1l_np, "w1r": w1r_np, "w2l": w2l_np, "w2r": w2r_np,
            "b1": b1_np, "b2": b2_np,
            "iota": iota_np, "ident": ident_np,
        })
    return nc, in_maps


def _run(x, edge_index, W1_l, b1_l, W1_r, W2_l, b2_l, W2_r, trace=False):
    global last_bass_results
    nc, in_maps = _prepare(x, edge_index, W1_l, b1_l, W1_r, W2_l, b2_l, W2_r)
    ncores = int(os.environ.get("KERNEL_DEBUG_NCORES", str(M)))
    res = bass_utils.run_bass_kernel_spmd(
        nc, in_maps[:ncores], core_ids=list(range(ncores)), trace=trace
    )
    if ncores < M:
        out = np.concatenate(
            [res.results[c]["out_shard"] for c in range(ncores)], axis=0
        )
        last_bass_results = res
        return out
    last_bass_results = res
    out = np.concatenate([res.results[c]["out_shard"] for c in range(M)], axis=0)
    return out


def kernel(x, edge_index, W1_l, b1_l, W1_r, W2_l, b2_l, W2_r):
    return _run(x, edge_index, W1_l, b1_l, W1_r, W2_l, b2_l, W2_r, trace=False)



# revision 5
# speedup vs baseline: 1.7037x; 1.7037x over previous
"""Two-layer GraphSAGE (mean aggregation) on 8 Trainium2 NeuronCores.

Strategy (1D graph/data parallelism, edges partitioned by dst):
  - Core c owns dst nodes [c*NP, (c+1)*NP), NP = N/8.
  - Host sorts edges by (core, src-chunk, dst-block); src split into 4 chunks
    of <=32768 rows so dma_gather's int16 indices can address them.
  - Device: per 128-edge tile, dma_gather pulls x[src] rows (edge-major,
    [128 edges x 64 feat]); a 0/1 selection matrix S (built on DVE from
    per-edge dst-slot ids vs an iota row) turns segment-sum into a PE matmul
    accumulated in PSUM per dst-block; partial sums for a block are
    accumulated across chunks in a resident SBUF accumulator.
  - Block tail: scale by 1/deg, PE-transpose, two accumulated matmuls
    (aggr @ W_l.T + x_own @ W_r.T) + bias.
  - x1 shards are AllGather'd across the 8 cores between layers.
  - Weights (64x64) are replicated; the full output is the concat of shards.
"""
import sys

sys.path.insert(0, "/opt/trn_rl_repo")

import numpy as np

import concourse.bass as bass
import concourse.bacc as bacc
import concourse.mybir as mybir
import concourse.tile as tile
from concourse import bass_utils

P = 128
D = 64
M = 8          # cores
CH = 32768     # src chunk rows (int16-addressable)
GT = 8         # tiles per dma_gather instruction (HW caps num_idxs at 1024)
WB = 4         # dst blocks per batched tail DMA write

F32 = mybir.dt.float32
I16 = mybir.dt.int16

import os
_STAGE = int(os.environ.get("KERNEL_DEBUG_STAGE", "3"))  # 1=L1, 2=L1+AG, 3=full
_NO_XOWNT = bool(int(os.environ.get("K_NO_XOWNT", "0")))
_NO_INVD = bool(int(os.environ.get("K_NO_INVD", "0")))
_ONE_MM = bool(int(os.environ.get("K_ONE_MM", "0")))
_NO_BIAS = bool(int(os.environ.get("K_NO_BIAS", "0")))
_NO_TAIL = bool(int(os.environ.get("K_NO_TAIL", "0")))
_NO_GATHER = bool(int(os.environ.get("K_NO_GATHER", "0")))
_SINGLE_PACKET = bool(int(os.environ.get("K_SINGLE_PACKET", "1")))
_NQUEUES = int(os.environ.get("K_NQUEUES", "4"))
_NO_SBUILD = bool(int(os.environ.get("K_NO_SBUILD", "0")))
_NO_MM = bool(int(os.environ.get("K_NO_MM", "0")))

last_bass_results = None  # test.py reads exec_time_ns off this

_prog_cache = {}


def _build_schedule(src, dst, N, NP):
    """Host-side edge partitioning. Returns per-core device input arrays and
    the (shared across cores) tile schedule."""
    E = src.shape[0]
    NB = -(-NP // P)                      # dst blocks per core
    NQ = -(-N // CH)                      # src chunks

    deg = np.bincount(dst, minlength=N).astype(np.int64)

    core_e = dst // NP
    r = dst - core_e * NP
    blk_e = r // P
    slot_e = (r % P).astype(np.float32)
    q_e = src // CH
    loc_e = (src - q_e * CH).astype(np.int16)
    assert loc_e.min() >= 0

    key = (core_e * NQ + q_e) * NB + blk_e
    order = np.argsort(key, kind="stable")
    ks = key[order]

    cnt = np.bincount(key, minlength=M * NQ * NB).reshape(M, NQ, NB)
    nt = -(-cnt.max(axis=0) // P)          # [NQ, NB] tiles per (q, B)
    NT = int(nt.sum())
    NI = NT * P

    toff = np.zeros(NQ * NB, np.int64)
    toff[1:] = np.cumsum(nt.reshape(-1))[:-1]
    toff = toff.reshape(NQ, NB)

    # per-edge flat position inside its core's index array
    change = np.empty(E, bool)
    change[0] = True
    change[1:] = ks[1:] != ks[:-1]
    run_start = np.flatnonzero(change)
    run_id = np.cumsum(change) - 1
    rank = np.arange(E) - run_start[run_id]
    kq = (ks // NB) % NQ
    kB = ks % NB
    pos = toff[kq, kB] * P + rank
    c_e = ks // (NQ * NB)

    idxflat = np.zeros((M, NI), np.int16)
    slotflat = np.full((M, NI), -1.0, np.float32)
    idxflat[c_e, pos] = loc_e[order]
    slotflat[c_e, pos] = slot_e[order]

    # dma_gather wrap: index i -> [i%16, i//16], replicated over 8 groups
    idx_w = np.ascontiguousarray(
        np.tile(idxflat.reshape(M, NI // 16, 16).transpose(0, 2, 1), (1, 8, 1))
    )
    slot_w = np.ascontiguousarray(slotflat.reshape(M, NT, P).transpose(0, 2, 1))

    degp = np.ones((M, NB * P), np.float32)
    degp[:, :NP] = np.maximum(deg.reshape(M, NP), 1)
    invd_w = np.ascontiguousarray(
        (1.0 / degp).reshape(M, NB, P).transpose(0, 2, 1).astype(np.float32)
    )

    # schedule: per chunk, the global tile range; groups of <= GT tiles
    groups = []        # (q, g0, g1)
    tile_meta = []     # per global tile t: (block, first_of_run, last_of_run)
    for q in range(NQ):
        for B in range(NB):
            n = int(nt[q, B])
            for j in range(n):
                tile_meta.append((B, j == 0, j == n - 1))
        t0 = int(toff[q, 0])
        t_end = t0 + int(nt[q].sum())
        g = t0
        while g < t_end:
            g1 = min(g + GT, t_end)
            groups.append((q, g, g1))
            g = g1

    chunk_rows = [min(CH, N - q * CH) for q in range(NQ)]
    sched = {
        "N": N, "NP": NP, "NB": NB, "NQ": NQ, "NT": NT, "NI": NI,
        "groups": groups, "tile_meta": tile_meta, "chunk_rows": chunk_rows,
    }
    return sched, idx_w, slot_w, invd_w


def _build_program(sched):
    N, NP, NB, NQ, NT, NI = (
        sched["N"], sched["NP"], sched["NB"], sched["NQ"], sched["NT"], sched["NI"]
    )
    groups, tile_meta, chunk_rows = (
        sched["groups"], sched["tile_meta"], sched["chunk_rows"]
    )

    nc = bacc.Bacc(
        "TRN2", num_devices=M, num_swdge_queues=_NQUEUES,
        dynamic_dma_scratch_size=int(os.environ.get("K_SCRATCH", "49152")),
    )

    x_store = nc.dram_tensor("x_store", [N, D], F32, kind="ExternalInput")
    x_ownT = nc.dram_tensor("x_ownT", [D, NB * P], F32, kind="ExternalInput")
    idx16 = nc.dram_tensor("idx16", [P, NI // 16], I16, kind="ExternalInput")
    slots = nc.dram_tensor("slots", [P, NT], F32, kind="ExternalInput")
    invd = nc.dram_tensor("invd", [P, NB], F32, kind="ExternalInput")
    w1l = nc.dram_tensor("w1l", [D, D], F32, kind="ExternalInput")
    w1r = nc.dram_tensor("w1r", [D, D], F32, kind="ExternalInput")
    w2l = nc.dram_tensor("w2l", [D, D], F32, kind="ExternalInput")
    w2r = nc.dram_tensor("w2r", [D, D], F32, kind="ExternalInput")
    b1 = nc.dram_tensor("b1", [P, D], F32, kind="ExternalInput")
    b2 = nc.dram_tensor("b2", [P, D], F32, kind="ExternalInput")
    iota_in = nc.dram_tensor("iota", [P, P], F32, kind="ExternalInput")
    ident_in = nc.dram_tensor("ident", [P, P], F32, kind="ExternalInput")
    out_shard = nc.dram_tensor("out_shard", [NP, D], F32, kind="ExternalOutput")

    with tile.TileContext(nc) as tc:
        with (
            tc.tile_pool(name="const", bufs=1) as cpool,
            tc.tile_pool(name="res", bufs=1) as rpool,
            tc.tile_pool(name="gpool", bufs=4) as gpool,
            tc.tile_pool(name="spool", bufs=3) as spool,
            tc.tile_pool(name="wpool", bufs=3) as wpool,
            tc.tile_pool(name="paggr", bufs=3, space="PSUM") as paggr,
            tc.tile_pool(name="ptr", bufs=2, space="PSUM") as ptr,
            tc.tile_pool(name="pout", bufs=2, space="PSUM") as pout,
            tc.tile_pool(name="dram", bufs=1, space="DRAM") as dram,
        ):
            iota_sb = cpool.tile([P, P], F32)
            nc.sync.dma_start(iota_sb[:], iota_in[:])
            ident_sb = cpool.tile([P, P], F32)
            nc.sync.dma_start(ident_sb[:], ident_in[:])
            wl_sb, wr_sb, bias_sb = [], [], []
            for i, wsrc in enumerate((w1l, w2l)):
                t = cpool.tile([D, D], F32, tag=f"wl{i}")
                nc.sync.dma_start(t[:], wsrc[:])
                wl_sb.append(t)
            for i, wsrc in enumerate((w1r, w2r)):
                t = cpool.tile([D, D], F32, tag=f"wr{i}")
                nc.sync.dma_start(t[:], wsrc[:])
                wr_sb.append(t)
            for i, bsrc in enumerate((b1, b2)):
                t = cpool.tile([P, D], F32, tag=f"bias{i}")
                nc.sync.dma_start(t[:], bsrc[:])
                bias_sb.append(t)
            invd_sb = cpool.tile([P, NB], F32)
            nc.sync.dma_start(invd_sb[:], invd[:])
            xoT_sb = cpool.tile([D, NB * P], F32)
            nc.sync.dma_start(xoT_sb[:], x_ownT[:])
            # resident gather indices / dst-slot ids (one big contiguous load
            # each instead of per-group strided loads)
            idx_res = cpool.tile([P, NI // 16], I16)
            nc.sync.dma_start(idx_res[:], idx16[:])
            slot_res = cpool.tile([P, NT], F32)
            nc.sync.dma_start(slot_res[:], slots[:])

            x1_sb = rpool.tile([P, NB * D], F32, tag="x1_sb")
            x1_shard = dram.tile([NP, D], F32)
            x1_full = dram.tile([N, D], F32)

            nlayers = 1 if _STAGE < 3 else 2
            for layer in range(nlayers):
                acc = rpool.tile([P, NB * D], F32, tag="acc")
                nc.vector.memset(acc[:], 0.0)

                cur_pag = {}  # B -> open PSUM accumulation tile (may span groups)
                _gq = 0
                for (q, g0, g1) in groups:
                    ntg = g1 - g0
                    ni = ntg * P
                    base = q * CH
                    rows = chunk_rows[q]
                    if layer == 0:
                        src_ap = x_store[base : base + rows, :]
                    else:
                        src_ap = x1_full[base : base + rows, :]

                    g = gpool.tile([P, ntg * D], F32, tag="g")
                    if _NO_GATHER:
                        nc.vector.memset(g[:], 1.0)
                    else:
                        nc.gpsimd.dma_gather(
                            out_ap=g[:].rearrange("p (t d) -> p t d", t=ntg, d=D),
                            in_ap=src_ap,
                            idxs_ap=idx_res[:, g0 * 8 : g0 * 8 + ni // 16],
                            num_idxs=ni,
                            num_idxs_reg=ni,
                            elem_size=D,
                            elem_step=D,
                            single_packet=_SINGLE_PACKET,
                            queue_num=_gq % _NQUEUES,
                        )
                        _gq += 1

                    # one batched one-hot build for the whole group:
                    # Sbig[e, t*128+s] = (slot[e, t] == s)
                    Sbig = spool.tile([P, ntg * P], F32, tag="S")
                    if _NO_SBUILD:
                        nc.vector.memset(Sbig[:], 0.0)
                    else:
                        nc.vector.tensor_tensor(
                            out=Sbig[:].rearrange(
                                "p (t s) -> p t s", t=ntg, s=P
                            ),
                            in0=slot_res[:, g0:g1].broadcast_to((P, ntg, P)),
                            in1=iota_sb[:]
                            .broadcast_to((P, P, ntg))
                            .transpose([0, 2, 1]),
                            op=mybir.AluOpType.is_equal,
                        )

                    for t in range(g0, g1):
                        B, first, last = tile_meta[t]
                        if _NO_MM:
                            continue
                        if first:
                            cur_pag[B] = paggr.tile(
                                [P, D], F32, name="pag", tag="pag"
                            )
                        pag = cur_pag[B]
                        nc.tensor.matmul(
                            pag[:], lhsT=Sbig[:, (t - g0) * P : (t - g0 + 1) * P],
                            rhs=g[:, (t - g0) * D : (t - g0 + 1) * D],
                            start=first, stop=last,
                        )
                        if last:
                            asl = acc[:, B * D : (B + 1) * D]
                            nc.vector.tensor_tensor(
                                out=asl, in0=asl, in1=pag[:],
                                op=mybir.AluOpType.add,
                            )

                for B in range(NB):
                    if _NO_TAIL:
                        rows = min(P, NP - B * P)
                        nc.sync.dma_start(
                            x1_shard[B * P : B * P + rows, :],
                            acc[:rows, B * D : (B + 1) * D],
                        )
                        if layer == 0:
                            nc.vector.tensor_copy(
                                x1_sb[:, B * D : (B + 1) * D],
                                acc[:, B * D : (B + 1) * D],
                            )
                        continue
                    rows = min(P, NP - B * P)
                    asl = acc[:, B * D : (B + 1) * D]
                    if not _NO_INVD:
                        nc.vector.tensor_scalar_mul(
                            asl, asl, invd_sb[:, B : B + 1]
                        )
                    pt = ptr.tile([D, P], F32, tag="ptr")
                    nc.tensor.transpose(pt[:], asl, ident_sb[:])
                    accT = wpool.tile([D, P], F32, tag="accT")
                    nc.vector.tensor_copy(accT[:], pt[:])
                    if _NO_XOWNT:
                        xT = accT[:]
                    elif layer == 0:
                        xT = xoT_sb[:, B * P : (B + 1) * P]
                    else:
                        pt2 = ptr.tile([D, P], F32, tag="ptr")
                        nc.tensor.transpose(
                            pt2[:], x1_sb[:, B * D : (B + 1) * D], ident_sb[:]
                        )
                        xTt = wpool.tile([D, P], F32, tag="xT")
                        nc.vector.tensor_copy(xTt[:], pt2[:])
                        xT = xTt[:]
                    po = pout.tile([P, D], F32)
                    nc.tensor.matmul(
                        po[:], lhsT=accT[:], rhs=wl_sb[layer][:],
                        start=True, stop=_ONE_MM,
                    )
                    if not _ONE_MM:
                        nc.tensor.matmul(
                            po[:], lhsT=xT, rhs=wr_sb[layer][:],
                            start=False, stop=True,
                        )
                    # L2 reuses acc slices as output staging (block B's acc is
                    # dead after its transpose)
                    dst_res = x1_sb if layer == 0 else acc
                    osl = dst_res[:, B * D : (B + 1) * D]
                    if _NO_BIAS:
                        nc.vector.tensor_copy(osl, po[:])
                    else:
                        nc.vector.tensor_tensor(
                            out=osl, in0=po[:], in1=bias_sb[layer][:],
                            op=mybir.AluOpType.add,
                        )
                    dst_dram = x1_shard if layer == 0 else out_shard
                    # batched write: flush every WB full blocks (or tail)
                    if B == NB - 1 or (B % WB == WB - 1 and (B + 1) * P <= NP):
                        b0 = (B // WB) * WB
                        nblk = B - b0 + 1
                        r0 = b0 * P
                        r1 = min(NP, (B + 1) * P)
                        if nblk > 1 and r1 == (B + 1) * P:
                            nc.sync.dma_start(
                                dst_dram[r0:r1, :].rearrange(
                                    "(j p) d -> p j d", p=P
                                ),
                                dst_res[:, b0 * D : (B + 1) * D].rearrange(
                                    "p (j d) -> p j d", d=D
                                ),
                            )
                        else:
                            for Bj in range(b0, B + 1):
                                rj0 = Bj * P
                                rj1 = min(NP, (Bj + 1) * P)
                                nc.sync.dma_start(
                                    dst_dram[rj0:rj1, :],
                                    dst_res[: rj1 - rj0, Bj * D : (Bj + 1) * D],
                                )

                if layer == 0 and _STAGE >= 2:
                    nc.gpsimd.collective_compute(
                        "AllGather",
                        mybir.AluOpType.bypass,
                        replica_groups=[list(range(M))],
                        ins=[x1_shard.opt()],
                        outs=[x1_full.opt()],
                    )
                if layer == 0 and _STAGE < 3:
                    # debug: emit x1 as the output so the program has a writer
                    for B in range(NB):
                        rows = min(P, NP - B * P)
                        nc.sync.dma_start(
                            out_shard[B * P : B * P + rows, :],
                            x1_sb[:rows, B * D : (B + 1) * D],
                        )

    nc.compile()
    return nc


def _prepare(x, edge_index, W1_l, b1_l, W1_r, W2_l, b2_l, W2_r):
    N, _D = x.shape
    assert _D == D and N % M == 0
    NP = N // M

    src = np.asarray(edge_index[0], dtype=np.int64)
    dst = np.asarray(edge_index[1], dtype=np.int64)

    sched, idx_w, slot_w, invd_w = _build_schedule(src, dst, N, NP)
    NB = sched["NB"]

    ck = (N, NP, sched["NT"], tuple(sched["groups"]),
          tuple(t for t in sched["tile_meta"]))
    import hashlib
    hk = hashlib.sha1(repr(ck).encode()).hexdigest()
    if hk not in _prog_cache:
        _prog_cache[hk] = _build_program(sched)
    nc = _prog_cache[hk]

    x = np.ascontiguousarray(np.asarray(x, np.float32))
    xoT = np.zeros((M, D, NB * P), np.float32)
    xr = x.reshape(M, NP, D)
    xoT[:, :, :NP] = xr.transpose(0, 2, 1)

    w1l_np = np.ascontiguousarray(np.asarray(W1_l, np.float32).T)
    w1r_np = np.ascontiguousarray(np.asarray(W1_r, np.float32).T)
    w2l_np = np.ascontiguousarray(np.asarray(W2_l, np.float32).T)
    w2r_np = np.ascontiguousarray(np.asarray(W2_r, np.float32).T)
    b1_np = np.ascontiguousarray(
        np.broadcast_to(np.asarray(b1_l, np.float32), (P, D))
    )
    b2_np = np.ascontiguousarray(
        np.broadcast_to(np.asarray(b2_l, np.float32), (P, D))
    )
    iota_np = np.ascontiguousarray(
        np.tile(np.arange(P, dtype=np.float32), (P, 1))
    )
    ident_np = np.eye(P, dtype=np.float32)

    in_maps = []
    for c in range(M):
        in_maps.append({
            "x_store": x,
            "x_ownT": np.ascontiguousarray(xoT[c]),
            "idx16": idx_w[c],
            "slots": slot_w[c],
            "invd": invd_w[c],
            "w1l": w1l_np, "w1r": w1r_np, "w2l": w2l_np, "w2r": w2r_np,
            "b1": b1_np, "b2": b2_np,
            "iota": iota_np, "ident": ident_np,
        })
    return nc, in_maps


def _run(x, edge_index, W1_l, b1_l, W1_r, W2_l, b2_l, W2_r, trace=False):
    global last_bass_results
    nc, in_maps = _prepare(x, edge_index, W1_l, b1_l, W1_r, W2_l, b2_l, W2_r)
    ncores = int(os.environ.get("KERNEL_DEBUG_NCORES", str(M)))
    res = bass_utils.run_bass_kernel_spmd(
        nc, in_maps[:ncores], core_ids=list(range(ncores)), trace=trace
    )
    if ncores < M:
        out = np.concatenate(
            [res.results[c]["out_shard"] for c in range(ncores)], axis=0
        )
        last_bass_results = res
        return out
    last_bass_results = res
    out = np.concatenate([res.results[c]["out_shard"] for c in range(M)], axis=0)
    return out


def kernel(x, edge_index, W1_l, b1_l, W1_r, W2_l, b2_l, W2_r):
    return _run(x, edge_index, W1_l, b1_l, W1_r, W2_l, b2_l, W2_r, trace=False)



# revision 8
# speedup vs baseline: 1.8422x; 1.0813x over previous
"""Two-layer GraphSAGE (mean aggregation) on 8 Trainium2 NeuronCores.

Strategy (1D graph/data parallelism, edges partitioned by dst):
  - Core c owns dst nodes [c*NP, (c+1)*NP), NP = N/8.
  - Host sorts edges by (core, src-chunk, dst-block); src split into 4 chunks
    of <=32768 rows so dma_gather's int16 indices can address them.
  - Device: per 128-edge tile, dma_gather pulls x[src] rows (edge-major,
    [128 edges x 64 feat]); a 0/1 selection matrix S (built on DVE from
    per-edge dst-slot ids vs an iota row) turns segment-sum into a PE matmul
    accumulated in PSUM per dst-block; partial sums for a block are
    accumulated across chunks in a resident SBUF accumulator.
  - Block tail: scale by 1/deg, PE-transpose, two accumulated matmuls
    (aggr @ W_l.T + x_own @ W_r.T) + bias.
  - x1 shards are AllGather'd across the 8 cores between layers.
  - Weights (64x64) are replicated; the full output is the concat of shards.
"""
import sys

sys.path.insert(0, "/opt/trn_rl_repo")

import numpy as np

import concourse.bass as bass
import concourse.bacc as bacc
import concourse.mybir as mybir
import concourse.tile as tile
from concourse import bass_utils

P = 128
D = 64
M = 8          # cores
CH = 32768     # src chunk rows (int16-addressable)
GT = 8         # tiles per dma_gather instruction (HW caps num_idxs at 1024)
WB = 4         # dst blocks per batched tail DMA write

F32 = mybir.dt.float32
BF16 = mybir.dt.bfloat16
I16 = mybir.dt.int16

import os
_STAGE = int(os.environ.get("KERNEL_DEBUG_STAGE", "3"))  # 1=L1, 2=L1+AG, 3=full
_NO_XOWNT = bool(int(os.environ.get("K_NO_XOWNT", "0")))
_NO_INVD = bool(int(os.environ.get("K_NO_INVD", "0")))
_ONE_MM = bool(int(os.environ.get("K_ONE_MM", "0")))
_NO_BIAS = bool(int(os.environ.get("K_NO_BIAS", "0")))
_NO_TAIL = bool(int(os.environ.get("K_NO_TAIL", "0")))
_NO_GATHER = bool(int(os.environ.get("K_NO_GATHER", "0")))
_SINGLE_PACKET = bool(int(os.environ.get("K_SINGLE_PACKET", "1")))
_NQUEUES = int(os.environ.get("K_NQUEUES", "4"))
_NO_SBUILD = bool(int(os.environ.get("K_NO_SBUILD", "0")))
_NO_MM = bool(int(os.environ.get("K_NO_MM", "0")))

last_bass_results = None  # test.py reads exec_time_ns off this

_prog_cache = {}


def _build_schedule(src, dst, N, NP):
    """Host-side edge partitioning. Returns per-core device input arrays and
    the (shared across cores) tile schedule."""
    E = src.shape[0]
    NB = -(-NP // P)                      # dst blocks per core
    NQ = -(-N // CH)                      # src chunks

    deg = np.bincount(dst, minlength=N).astype(np.int64)

    core_e = dst // NP
    r = dst - core_e * NP
    blk_e = r // P
    slot_e = (r % P).astype(np.float32)
    q_e = src // CH
    loc_e = (src - q_e * CH).astype(np.int16)
    assert loc_e.min() >= 0

    key = (core_e * NQ + q_e) * NB + blk_e
    order = np.argsort(key, kind="stable")
    ks = key[order]

    cnt = np.bincount(key, minlength=M * NQ * NB).reshape(M, NQ, NB)
    nt = -(-cnt.max(axis=0) // P)          # [NQ, NB] tiles per (q, B)
    NT = int(nt.sum())
    NI = NT * P

    toff = np.zeros(NQ * NB, np.int64)
    toff[1:] = np.cumsum(nt.reshape(-1))[:-1]
    toff = toff.reshape(NQ, NB)

    # per-edge flat position inside its core's index array
    change = np.empty(E, bool)
    change[0] = True
    change[1:] = ks[1:] != ks[:-1]
    run_start = np.flatnonzero(change)
    run_id = np.cumsum(change) - 1
    rank = np.arange(E) - run_start[run_id]
    kq = (ks // NB) % NQ
    kB = ks % NB
    pos = toff[kq, kB] * P + rank
    c_e = ks // (NQ * NB)

    idxflat = np.zeros((M, NI), np.int16)
    slotflat = np.full((M, NI), -1.0, np.float32)
    idxflat[c_e, pos] = loc_e[order]
    slotflat[c_e, pos] = slot_e[order]

    # dma_gather wrap: index i -> [i%16, i//16], replicated over 8 groups
    idx_w = np.ascontiguousarray(
        np.tile(idxflat.reshape(M, NI // 16, 16).transpose(0, 2, 1), (1, 8, 1))
    )
    slot_w = np.ascontiguousarray(slotflat.reshape(M, NT, P).transpose(0, 2, 1))

    degp = np.ones((M, NB * P), np.float32)
    degp[:, :NP] = np.maximum(deg.reshape(M, NP), 1)
    invd_w = np.ascontiguousarray(
        (1.0 / degp).reshape(M, NB, P).transpose(0, 2, 1).astype(np.float32)
    )

    # schedule: per chunk, the global tile range; groups of <= GT tiles
    groups = []        # (q, g0, g1)
    tile_meta = []     # per global tile t: (block, first_of_run, last_of_run)
    for q in range(NQ):
        for B in range(NB):
            n = int(nt[q, B])
            for j in range(n):
                tile_meta.append((B, j == 0, j == n - 1))
        t0 = int(toff[q, 0])
        t_end = t0 + int(nt[q].sum())
        g = t0
        while g < t_end:
            g1 = min(g + GT, t_end)
            groups.append((q, g, g1))
            g = g1

    chunk_rows = [min(CH, N - q * CH) for q in range(NQ)]
    sched = {
        "N": N, "NP": NP, "NB": NB, "NQ": NQ, "NT": NT, "NI": NI,
        "groups": groups, "tile_meta": tile_meta, "chunk_rows": chunk_rows,
    }
    return sched, idx_w, slot_w, invd_w


def _build_program(sched):
    N, NP, NB, NQ, NT, NI = (
        sched["N"], sched["NP"], sched["NB"], sched["NQ"], sched["NT"], sched["NI"]
    )
    groups, tile_meta, chunk_rows = (
        sched["groups"], sched["tile_meta"], sched["chunk_rows"]
    )

    nc = bacc.Bacc(
        "TRN2", num_devices=M, num_swdge_queues=_NQUEUES,
        dynamic_dma_scratch_size=int(os.environ.get("K_SCRATCH", "49152")),
    )

    x_store = nc.dram_tensor("x_store", [N, D], F32, kind="ExternalInput")
    x_ownT = nc.dram_tensor("x_ownT", [D, NB * P], F32, kind="ExternalInput")
    idx16 = nc.dram_tensor("idx16", [P, NI // 16], I16, kind="ExternalInput")
    slots = nc.dram_tensor("slots", [P, NT], F32, kind="ExternalInput")
    invd = nc.dram_tensor("invd", [P, NB], F32, kind="ExternalInput")
    w1l = nc.dram_tensor("w1l", [D, D], F32, kind="ExternalInput")
    w1r = nc.dram_tensor("w1r", [D, D], F32, kind="ExternalInput")
    w2l = nc.dram_tensor("w2l", [D, D], F32, kind="ExternalInput")
    w2r = nc.dram_tensor("w2r", [D, D], F32, kind="ExternalInput")
    b1 = nc.dram_tensor("b1", [P, D], F32, kind="ExternalInput")
    b2 = nc.dram_tensor("b2", [P, D], F32, kind="ExternalInput")
    iota_in = nc.dram_tensor("iota", [P, P], F32, kind="ExternalInput")
    ident_in = nc.dram_tensor("ident", [P, P], F32, kind="ExternalInput")
    out_shard = nc.dram_tensor("out_shard", [NP, D], F32, kind="ExternalOutput")

    with tile.TileContext(nc) as tc:
        with (
            tc.tile_pool(name="const", bufs=1) as cpool,
            tc.tile_pool(name="res", bufs=1) as rpool,
            tc.tile_pool(name="gpool", bufs=4) as gpool,
            tc.tile_pool(name="gbpool", bufs=4) as gbpool,
            tc.tile_pool(name="spool", bufs=3) as spool,
            tc.tile_pool(name="wpool", bufs=3) as wpool,
            tc.tile_pool(name="paggr", bufs=3, space="PSUM") as paggr,
            tc.tile_pool(name="ptr", bufs=2, space="PSUM") as ptr,
            tc.tile_pool(name="pout", bufs=2, space="PSUM") as pout,
            tc.tile_pool(name="dram", bufs=1, space="DRAM") as dram,
        ):
            iota_sb = cpool.tile([P, P], F32)
            nc.sync.dma_start(iota_sb[:], iota_in[:])
            ident_sb = cpool.tile([P, P], F32)
            nc.sync.dma_start(ident_sb[:], ident_in[:])
            wl_sb, wr_sb, bias_sb = [], [], []
            for i, wsrc in enumerate((w1l, w2l)):
                t = cpool.tile([D, D], F32, tag=f"wl{i}")
                nc.sync.dma_start(t[:], wsrc[:])
                wl_sb.append(t)
            for i, wsrc in enumerate((w1r, w2r)):
                t = cpool.tile([D, D], F32, tag=f"wr{i}")
                nc.sync.dma_start(t[:], wsrc[:])
                wr_sb.append(t)
            for i, bsrc in enumerate((b1, b2)):
                t = cpool.tile([P, D], F32, tag=f"bias{i}")
                nc.sync.dma_start(t[:], bsrc[:])
                bias_sb.append(t)
            invd_sb = cpool.tile([P, NB], F32)
            nc.sync.dma_start(invd_sb[:], invd[:])
            xoT_sb = cpool.tile([D, NB * P], F32)
            nc.sync.dma_start(xoT_sb[:], x_ownT[:])
            # resident gather indices / dst-slot ids (one big contiguous load
            # each instead of per-group strided loads)
            idx_res = cpool.tile([P, NI // 16], I16)
            nc.sync.dma_start(idx_res[:], idx16[:])
            slot_res = cpool.tile([P, NT], F32)
            nc.sync.dma_start(slot_res[:], slots[:])

            x1_sb = rpool.tile([P, NB * D], F32, tag="x1_sb")
            x1_shard = dram.tile([NP, D], F32)
            x1_full = dram.tile([N, D], F32)

            nlayers = 1 if _STAGE < 3 else 2
            for layer in range(nlayers):
                acc = rpool.tile([P, NB * D], F32, tag="acc")
                nc.vector.memset(acc[:], 0.0)

                cur_pag = {}  # B -> open PSUM accumulation tile (may span groups)
                _gq = 0
                for (q, g0, g1) in groups:
                    ntg = g1 - g0
                    ni = ntg * P
                    base = q * CH
                    rows = chunk_rows[q]
                    if layer == 0:
                        src_ap = x_store[base : base + rows, :]
                    else:
                        src_ap = x1_full[base : base + rows, :]

                    g = gpool.tile([P, ntg * D], F32, tag="g")
                    if _NO_GATHER:
                        nc.vector.memset(g[:], 1.0)
                    else:
                        nc.gpsimd.dma_gather(
                            out_ap=g[:].rearrange("p (t d) -> p t d", t=ntg, d=D),
                            in_ap=src_ap,
                            idxs_ap=idx_res[:, g0 * 8 : g0 * 8 + ni // 16],
                            num_idxs=ni,
                            num_idxs_reg=ni,
                            elem_size=D,
                            elem_step=D,
                            single_packet=_SINGLE_PACKET,
                            queue_num=_gq % _NQUEUES,
                        )
                        _gq += 1

                    # one batched one-hot build for the whole group:
                    # Sbig[e, t*128+s] = (slot[e, t] == s), emitted as bf16
                    Sbig = spool.tile([P, ntg * P], BF16, tag="S")
                    if _NO_SBUILD:
                        nc.vector.memset(Sbig[:], 0.0)
                    else:
                        nc.vector.tensor_tensor(
                            out=Sbig[:].rearrange(
                                "p (t s) -> p t s", t=ntg, s=P
                            ),
                            in0=slot_res[:, g0:g1].broadcast_to((P, ntg, P)),
                            in1=iota_sb[:]
                            .broadcast_to((P, P, ntg))
                            .transpose([0, 2, 1]),
                            op=mybir.AluOpType.is_equal,
                        )
                    # bf16 copy of the gathered tile (scalar engine is idle)
                    gb = gbpool.tile([P, ntg * D], BF16, tag="gb")
                    nc.scalar.activation(
                        gb[:], g[:], mybir.ActivationFunctionType.Copy
                    )

                    for t in range(g0, g1):
                        B, first, last = tile_meta[t]
                        if _NO_MM:
                            continue
                        if first:
                            cur_pag[B] = paggr.tile(
                                [P, D], F32, name="pag", tag="pag"
                            )
                        pag = cur_pag[B]
                        nc.tensor.matmul(
                            pag[:], lhsT=Sbig[:, (t - g0) * P : (t - g0 + 1) * P],
                            rhs=gb[:, (t - g0) * D : (t - g0 + 1) * D],
                            start=first, stop=last,
                        )
                        if last:
                            asl = acc[:, B * D : (B + 1) * D]
                            nc.vector.tensor_tensor(
                                out=asl, in0=asl, in1=pag[:],
                                op=mybir.AluOpType.add,
                            )

                for B in range(NB):
                    if _NO_TAIL:
                        rows = min(P, NP - B * P)
                        nc.sync.dma_start(
                            x1_shard[B * P : B * P + rows, :],
                            acc[:rows, B * D : (B + 1) * D],
                        )
                        if layer == 0:
                            nc.vector.tensor_copy(
                                x1_sb[:, B * D : (B + 1) * D],
                                acc[:, B * D : (B + 1) * D],
                            )
                        continue
                    rows = min(P, NP - B * P)
                    asl = acc[:, B * D : (B + 1) * D]
                    if not _NO_INVD:
                        nc.vector.tensor_scalar_mul(
                            asl, asl, invd_sb[:, B : B + 1]
                        )
                    pt = ptr.tile([D, P], F32, tag="ptr")
                    nc.tensor.transpose(pt[:], asl, ident_sb[:])
                    accT = wpool.tile([D, P], F32, tag="accT")
                    nc.vector.tensor_copy(accT[:], pt[:])
                    if _NO_XOWNT:
                        xT = accT[:]
                    elif layer == 0:
                        xT = xoT_sb[:, B * P : (B + 1) * P]
                    else:
                        pt2 = ptr.tile([D, P], F32, tag="ptr")
                        nc.tensor.transpose(
                            pt2[:], x1_sb[:, B * D : (B + 1) * D], ident_sb[:]
                        )
                        xTt = wpool.tile([D, P], F32, tag="xT")
                        nc.vector.tensor_copy(xTt[:], pt2[:])
                        xT = xTt[:]
                    po = pout.tile([P, D], F32)
                    nc.tensor.matmul(
                        po[:], lhsT=accT[:], rhs=wl_sb[layer][:],
                        start=True, stop=_ONE_MM,
                    )
                    if not _ONE_MM:
                        nc.tensor.matmul(
                            po[:], lhsT=xT, rhs=wr_sb[layer][:],
                            start=False, stop=True,
                        )
                    # L2 reuses acc slices as output staging (block B's acc is
                    # dead after its transpose)
                    dst_res = x1_sb if layer == 0 else acc
                    osl = dst_res[:, B * D : (B + 1) * D]
                    if _NO_BIAS:
                        nc.vector.tensor_copy(osl, po[:])
                    else:
                        nc.vector.tensor_tensor(
                            out=osl, in0=po[:], in1=bias_sb[layer][:],
                            op=mybir.AluOpType.add,
                        )
                    dst_dram = x1_shard if layer == 0 else out_shard
                    # batched write: flush every WB full blocks (or tail)
                    if B == NB - 1 or (B % WB == WB - 1 and (B + 1) * P <= NP):
                        b0 = (B // WB) * WB
                        nblk = B - b0 + 1
                        r0 = b0 * P
                        r1 = min(NP, (B + 1) * P)
                        if nblk > 1 and r1 == (B + 1) * P:
                            nc.sync.dma_start(
                                dst_dram[r0:r1, :].rearrange(
                                    "(j p) d -> p j d", p=P
                                ),
                                dst_res[:, b0 * D : (B + 1) * D].rearrange(
                                    "p (j d) -> p j d", d=D
                                ),
                            )
                        else:
                            for Bj in range(b0, B + 1):
                                rj0 = Bj * P
                                rj1 = min(NP, (Bj + 1) * P)
                                nc.sync.dma_start(
                                    dst_dram[rj0:rj1, :],
                                    dst_res[: rj1 - rj0, Bj * D : (Bj + 1) * D],
                                )

                if layer == 0 and _STAGE >= 2:
                    nc.gpsimd.collective_compute(
                        "AllGather",
                        mybir.AluOpType.bypass,
                        replica_groups=[list(range(M))],
                        ins=[x1_shard.opt()],
                        outs=[x1_full.opt()],
                    )
                if layer == 0 and _STAGE < 3:
                    # debug: emit x1 as the output so the program has a writer
                    for B in range(NB):
                        rows = min(P, NP - B * P)
                        nc.sync.dma_start(
                            out_shard[B * P : B * P + rows, :],
                            x1_sb[:rows, B * D : (B + 1) * D],
                        )

    nc.compile()
    return nc


def _prepare(x, edge_index, W1_l, b1_l, W1_r, W2_l, b2_l, W2_r):
    N, _D = x.shape
    assert _D == D and N % M == 0
    NP = N // M

    src = np.asarray(edge_index[0], dtype=np.int64)
    dst = np.asarray(edge_index[1], dtype=np.int64)

    sched, idx_w, slot_w, invd_w = _build_schedule(src, dst, N, NP)
    NB = sched["NB"]

    ck = (N, NP, sched["NT"], tuple(sched["groups"]),
          tuple(t for t in sched["tile_meta"]))
    import hashlib
    hk = hashlib.sha1(repr(ck).encode()).hexdigest()
    if hk not in _prog_cache:
        _prog_cache[hk] = _build_program(sched)
    nc = _prog_cache[hk]

    x = np.ascontiguousarray(np.asarray(x, np.float32))
    xoT = np.zeros((M, D, NB * P), np.float32)
    xr = x.reshape(M, NP, D)
    xoT[:, :, :NP] = xr.transpose(0, 2, 1)

    w1l_np = np.ascontiguousarray(np.asarray(W1_l, np.float32).T)
    w1r_np = np.ascontiguousarray(np.asarray(W1_r, np.float32).T)
    w2l_np = np.ascontiguousarray(np.asarray(W2_l, np.float32).T)
    w2r_np = np.ascontiguousarray(np.asarray(W2_r, np.float32).T)
    b1_np = np.ascontiguousarray(
        np.broadcast_to(np.asarray(b1_l, np.float32), (P, D))
    )
    b2_np = np.ascontiguousarray(
        np.broadcast_to(np.asarray(b2_l, np.float32), (P, D))
    )
    iota_np = np.ascontiguousarray(
        np.tile(np.arange(P, dtype=np.float32), (P, 1))
    )
    ident_np = np.eye(P, dtype=np.float32)

    in_maps = []
    for c in range(M):
        in_maps.append({
            "x_store": x,
            "x_ownT": np.ascontiguousarray(xoT[c]),
            "idx16": idx_w[c],
            "slots": slot_w[c],
            "invd": invd_w[c],
            "w1l": w1l_np, "w1r": w1r_np, "w2l": w2l_np, "w2r": w2r_np,
            "b1": b1_np, "b2": b2_np,
            "iota": iota_np, "ident": ident_np,
        })
    return nc, in_maps


def _run(x, edge_index, W1_l, b1_l, W1_r, W2_l, b2_l, W2_r, trace=False):
    global last_bass_results
    nc, in_maps = _prepare(x, edge_index, W1_l, b1_l, W1_r, W2_l, b2_l, W2_r)
    ncores = int(os.environ.get("KERNEL_DEBUG_NCORES", str(M)))
    res = bass_utils.run_bass_kernel_spmd(
        nc, in_maps[:ncores], core_ids=list(range(ncores)), trace=trace
    )
    if ncores < M:
        out = np.concatenate(
            [res.results[c]["out_shard"] for c in range(ncores)], axis=0
        )
        last_bass_results = res
        return out
    last_bass_results = res
    out = np.concatenate([res.results[c]["out_shard"] for c in range(M)], axis=0)
    return out


def kernel(x, edge_index, W1_l, b1_l, W1_r, W2_l, b2_l, W2_r):
    return _run(x, edge_index, W1_l, b1_l, W1_r, W2_l, b2_l, W2_r, trace=False)



# revision 10
# speedup vs baseline: 2.2005x; 1.1945x over previous
"""Two-layer GraphSAGE (mean aggregation) on 8 Trainium2 NeuronCores.

Strategy (1D graph/data parallelism, edges partitioned by dst):
  - Core c owns dst nodes [c*NP, (c+1)*NP), NP = N/8.
  - Host sorts edges by (core, src-chunk, dst-block); src split into 4 chunks
    of <=32768 rows so dma_gather's int16 indices can address them.
  - Device: per 128-edge tile, dma_gather pulls x[src] rows (edge-major,
    [128 edges x 64 feat]); a 0/1 selection matrix S (built on DVE from
    per-edge dst-slot ids vs an iota row) turns segment-sum into a PE matmul
    accumulated in PSUM per dst-block; partial sums for a block are
    accumulated across chunks in a resident SBUF accumulator.
  - Block tail: scale by 1/deg, PE-transpose, two accumulated matmuls
    (aggr @ W_l.T + x_own @ W_r.T) + bias.
  - x1 shards are AllGather'd across the 8 cores between layers.
  - Weights (64x64) are replicated; the full output is the concat of shards.
"""
import sys

sys.path.insert(0, "/opt/trn_rl_repo")

import numpy as np

import concourse.bass as bass
import concourse.bacc as bacc
import concourse.mybir as mybir
import concourse.tile as tile
from concourse import bass_utils

P = 128
D = 64
M = 8          # cores
CH = 32768     # src chunk rows (int16-addressable)
GT = 8         # tiles per dma_gather instruction (HW caps num_idxs at 1024)
WB = 4         # dst blocks per batched tail DMA write

F32 = mybir.dt.float32
BF16 = mybir.dt.bfloat16
I16 = mybir.dt.int16

import os
_STAGE = int(os.environ.get("KERNEL_DEBUG_STAGE", "3"))  # 1=L1, 2=L1+AG, 3=full
_NO_XOWNT = bool(int(os.environ.get("K_NO_XOWNT", "0")))
_NO_INVD = bool(int(os.environ.get("K_NO_INVD", "0")))
_ONE_MM = bool(int(os.environ.get("K_ONE_MM", "0")))
_NO_BIAS = bool(int(os.environ.get("K_NO_BIAS", "0")))
_NO_TAIL = bool(int(os.environ.get("K_NO_TAIL", "0")))
_NO_GATHER = bool(int(os.environ.get("K_NO_GATHER", "0")))
_SINGLE_PACKET = bool(int(os.environ.get("K_SINGLE_PACKET", "1")))
_NQUEUES = int(os.environ.get("K_NQUEUES", "4"))
_NO_SBUILD = bool(int(os.environ.get("K_NO_SBUILD", "0")))
_NO_MM = bool(int(os.environ.get("K_NO_MM", "0")))

last_bass_results = None  # test.py reads exec_time_ns off this

_prog_cache = {}


def _build_schedule(src, dst, N, NP):
    """Host-side edge partitioning. Returns per-core device input arrays and
    the (shared across cores) tile schedule."""
    E = src.shape[0]
    NB = -(-NP // P)                      # dst blocks per core
    NQ = -(-N // CH)                      # src chunks

    deg = np.bincount(dst, minlength=N).astype(np.int64)

    core_e = dst // NP
    r = dst - core_e * NP
    blk_e = r // P
    slot_e = (r % P).astype(np.float32)
    q_e = src // CH
    loc_e = (src - q_e * CH).astype(np.int16)
    assert loc_e.min() >= 0

    key = (core_e * NQ + q_e) * NB + blk_e
    order = np.argsort(key, kind="stable")
    ks = key[order]

    cnt = np.bincount(key, minlength=M * NQ * NB).reshape(M, NQ, NB)
    nt = -(-cnt.max(axis=0) // P)          # [NQ, NB] tiles per (q, B)
    NT = int(nt.sum())
    NI = NT * P

    toff = np.zeros(NQ * NB, np.int64)
    toff[1:] = np.cumsum(nt.reshape(-1))[:-1]
    toff = toff.reshape(NQ, NB)

    # per-edge flat position inside its core's index array
    change = np.empty(E, bool)
    change[0] = True
    change[1:] = ks[1:] != ks[:-1]
    run_start = np.flatnonzero(change)
    run_id = np.cumsum(change) - 1
    rank = np.arange(E) - run_start[run_id]
    kq = (ks // NB) % NQ
    kB = ks % NB
    pos = toff[kq, kB] * P + rank
    c_e = ks // (NQ * NB)

    idxflat = np.zeros((M, NI), np.int16)
    slotflat = np.full((M, NI), -1.0, np.float32)
    idxflat[c_e, pos] = loc_e[order]
    slotflat[c_e, pos] = slot_e[order]

    # dma_gather wrap: index i -> [i%16, i//16], replicated over 8 groups
    idx_w = np.ascontiguousarray(
        np.tile(idxflat.reshape(M, NI // 16, 16).transpose(0, 2, 1), (1, 8, 1))
    )
    slot_w = np.ascontiguousarray(slotflat.reshape(M, NT, P).transpose(0, 2, 1))

    degp = np.ones((M, NB * P), np.float32)
    degp[:, :NP] = np.maximum(deg.reshape(M, NP), 1)
    invd_w = np.ascontiguousarray(
        (1.0 / degp).reshape(M, NB, P).transpose(0, 2, 1).astype(np.float32)
    )

    # schedule: per chunk, the global tile range; groups of <= GT tiles
    groups = []        # (q, g0, g1)
    tile_meta = []     # per global tile t: (block, first_of_run, last_of_run)
    for q in range(NQ):
        for B in range(NB):
            n = int(nt[q, B])
            for j in range(n):
                tile_meta.append((B, j == 0, j == n - 1))
        t0 = int(toff[q, 0])
        t_end = t0 + int(nt[q].sum())
        g = t0
        while g < t_end:
            g1 = min(g + GT, t_end)
            groups.append((q, g, g1))
            g = g1

    chunk_rows = [min(CH, N - q * CH) for q in range(NQ)]
    sched = {
        "N": N, "NP": NP, "NB": NB, "NQ": NQ, "NT": NT, "NI": NI,
        "groups": groups, "tile_meta": tile_meta, "chunk_rows": chunk_rows,
    }
    return sched, idx_w, slot_w, invd_w


def _build_program(sched):
    N, NP, NB, NQ, NT, NI = (
        sched["N"], sched["NP"], sched["NB"], sched["NQ"], sched["NT"], sched["NI"]
    )
    groups, tile_meta, chunk_rows = (
        sched["groups"], sched["tile_meta"], sched["chunk_rows"]
    )

    nc = bacc.Bacc(
        "TRN2", num_devices=M, num_swdge_queues=_NQUEUES,
        dynamic_dma_scratch_size=int(os.environ.get("K_SCRATCH", "49152")),
    )

    x_store = nc.dram_tensor("x_store", [N, D], F32, kind="ExternalInput")
    x_ownT = nc.dram_tensor("x_ownT", [D, NB * P], F32, kind="ExternalInput")
    idx16 = nc.dram_tensor("idx16", [P, NI // 16], I16, kind="ExternalInput")
    slots = nc.dram_tensor("slots", [P, NT], F32, kind="ExternalInput")
    invd = nc.dram_tensor("invd", [P, NB], F32, kind="ExternalInput")
    w1l = nc.dram_tensor("w1l", [D, D], F32, kind="ExternalInput")
    w1r = nc.dram_tensor("w1r", [D, D], F32, kind="ExternalInput")
    w2l = nc.dram_tensor("w2l", [D, D], F32, kind="ExternalInput")
    w2r = nc.dram_tensor("w2r", [D, D], F32, kind="ExternalInput")
    b1 = nc.dram_tensor("b1", [P, D], F32, kind="ExternalInput")
    b2 = nc.dram_tensor("b2", [P, D], F32, kind="ExternalInput")
    iota_in = nc.dram_tensor("iota", [P, P], F32, kind="ExternalInput")
    ident_in = nc.dram_tensor("ident", [P, P], F32, kind="ExternalInput")
    out_shard = nc.dram_tensor("out_shard", [NP, D], F32, kind="ExternalOutput")

    with tile.TileContext(nc) as tc:
        with (
            tc.tile_pool(name="const", bufs=1) as cpool,
            tc.tile_pool(name="res", bufs=1) as rpool,
            tc.tile_pool(name="gpool", bufs=4) as gpool,
            tc.tile_pool(name="gbpool", bufs=4) as gbpool,
            tc.tile_pool(name="spool", bufs=3) as spool,
            tc.tile_pool(name="wpool", bufs=3) as wpool,
            tc.tile_pool(name="paggr", bufs=3, space="PSUM") as paggr,
            tc.tile_pool(name="ptr", bufs=2, space="PSUM") as ptr,
            tc.tile_pool(name="pout", bufs=2, space="PSUM") as pout,
            tc.tile_pool(name="dram", bufs=1, space="DRAM") as dram,
        ):
            iota_sb = cpool.tile([P, P], F32)
            nc.sync.dma_start(iota_sb[:], iota_in[:])
            ident_sb = cpool.tile([P, P], F32)
            nc.sync.dma_start(ident_sb[:], ident_in[:])
            wl_sb, wr_sb, bias_sb = [], [], []
            for i, wsrc in enumerate((w1l, w2l)):
                t = cpool.tile([D, D], F32, tag=f"wl{i}")
                nc.sync.dma_start(t[:], wsrc[:])
                wl_sb.append(t)
            for i, wsrc in enumerate((w1r, w2r)):
                t = cpool.tile([D, D], F32, tag=f"wr{i}")
                nc.sync.dma_start(t[:], wsrc[:])
                wr_sb.append(t)
            for i, bsrc in enumerate((b1, b2)):
                t = cpool.tile([P, D], F32, tag=f"bias{i}")
                nc.sync.dma_start(t[:], bsrc[:])
                bias_sb.append(t)
            invd_sb = cpool.tile([P, NB], F32)
            nc.sync.dma_start(invd_sb[:], invd[:])
            xoT_sb = cpool.tile([D, NB * P], F32)
            nc.sync.dma_start(xoT_sb[:], x_ownT[:])
            # resident gather indices / dst-slot ids (one big contiguous load
            # each instead of per-group strided loads)
            idx_res = cpool.tile([P, NI // 16], I16)
            nc.sync.dma_start(idx_res[:], idx16[:])
            slot_res = cpool.tile([P, NT], F32)
            nc.sync.dma_start(slot_res[:], slots[:])

            x1_sb = rpool.tile([P, NB * D], F32, tag="x1_sb")
            x1_shard = dram.tile([NP, D], F32)
            x1_full = dram.tile([N, D], F32)

            nlayers = 1 if _STAGE < 3 else 2
            for layer in range(nlayers):
                acc = rpool.tile([P, NB * D], F32, tag="acc")
                nc.vector.memset(acc[:], 0.0)

                cur_pag = {}  # B -> open PSUM accumulation tile (may span groups)
                _gq = 0
                for (q, g0, g1) in groups:
                    ntg = g1 - g0
                    ni = ntg * P
                    base = q * CH
                    rows = chunk_rows[q]
                    if layer == 0:
                        src_ap = x_store[base : base + rows, :]
                    else:
                        src_ap = x1_full[base : base + rows, :]

                    g = gpool.tile([P, ntg * D], F32, tag="g")
                    if _NO_GATHER:
                        nc.vector.memset(g[:], 1.0)
                    else:
                        nc.gpsimd.dma_gather(
                            out_ap=g[:].rearrange("p (t d) -> p t d", t=ntg, d=D),
                            in_ap=src_ap,
                            idxs_ap=idx_res[:, g0 * 8 : g0 * 8 + ni // 16],
                            num_idxs=ni,
                            num_idxs_reg=ni,
                            elem_size=D,
                            elem_step=D,
                            single_packet=_SINGLE_PACKET,
                            queue_num=_gq % _NQUEUES,
                        )
                        _gq += 1

                    # one batched one-hot build for the whole group:
                    # Sbig[e, t*128+s] = (slot[e, t] == s), emitted as bf16
                    Sbig = spool.tile([P, ntg * P], BF16, tag="S")
                    if _NO_SBUILD:
                        pass
                    else:
                        nc.vector.tensor_tensor(
                            out=Sbig[:].rearrange(
                                "p (t s) -> p t s", t=ntg, s=P
                            ),
                            in0=slot_res[:, g0:g1].broadcast_to((P, ntg, P)),
                            in1=iota_sb[:]
                            .broadcast_to((P, P, ntg))
                            .transpose([0, 2, 1]),
                            op=mybir.AluOpType.is_equal,
                        )
                    # bf16 copy of the gathered tile (scalar engine is idle)
                    gb = gbpool.tile([P, ntg * D], BF16, tag="gb")
                    if not _NO_MM:
                        nc.scalar.activation(
                            gb[:], g[:], mybir.ActivationFunctionType.Copy
                        )

                    for t in range(g0, g1):
                        B, first, last = tile_meta[t]
                        if _NO_MM:
                            continue
                        if first:
                            cur_pag[B] = paggr.tile(
                                [P, D], F32, name="pag", tag="pag"
                            )
                        pag = cur_pag[B]
                        nc.tensor.matmul(
                            pag[:], lhsT=Sbig[:, (t - g0) * P : (t - g0 + 1) * P],
                            rhs=gb[:, (t - g0) * D : (t - g0 + 1) * D],
                            start=first, stop=last,
                        )
                        if last:
                            asl = acc[:, B * D : (B + 1) * D]
                            nc.vector.tensor_tensor(
                                out=asl, in0=asl, in1=pag[:],
                                op=mybir.AluOpType.add,
                            )

                for B in range(NB):
                    if _NO_TAIL:
                        rows = min(P, NP - B * P)
                        nc.sync.dma_start(
                            x1_shard[B * P : B * P + rows, :],
                            acc[:rows, B * D : (B + 1) * D],
                        )
                        if layer == 0:
                            nc.vector.tensor_copy(
                                x1_sb[:, B * D : (B + 1) * D],
                                acc[:, B * D : (B + 1) * D],
                            )
                        continue
                    rows = min(P, NP - B * P)
                    asl = acc[:, B * D : (B + 1) * D]
                    if not _NO_INVD:
                        nc.vector.tensor_scalar_mul(
                            asl, asl, invd_sb[:, B : B + 1]
                        )
                    pt = ptr.tile([D, P], F32, tag="ptr")
                    nc.tensor.transpose(pt[:], asl, ident_sb[:])
                    accT = wpool.tile([D, P], F32, tag="accT")
                    nc.vector.tensor_copy(accT[:], pt[:])
                    if _NO_XOWNT:
                        xT = accT[:]
                    elif layer == 0:
                        xT = xoT_sb[:, B * P : (B + 1) * P]
                    else:
                        pt2 = ptr.tile([D, P], F32, tag="ptr")
                        nc.tensor.transpose(
                            pt2[:], x1_sb[:, B * D : (B + 1) * D], ident_sb[:]
                        )
                        xTt = wpool.tile([D, P], F32, tag="xT")
                        nc.vector.tensor_copy(xTt[:], pt2[:])
                        xT = xTt[:]
                    po = pout.tile([P, D], F32)
                    nc.tensor.matmul(
                        po[:], lhsT=accT[:], rhs=wl_sb[layer][:],
                        start=True, stop=_ONE_MM,
                    )
                    if not _ONE_MM:
                        nc.tensor.matmul(
                            po[:], lhsT=xT, rhs=wr_sb[layer][:],
                            start=False, stop=True,
                        )
                    # L2 reuses acc slices as output staging (block B's acc is
                    # dead after its transpose)
                    dst_res = x1_sb if layer == 0 else acc
                    osl = dst_res[:, B * D : (B + 1) * D]
                    if _NO_BIAS:
                        nc.vector.tensor_copy(osl, po[:])
                    else:
                        nc.vector.tensor_tensor(
                            out=osl, in0=po[:], in1=bias_sb[layer][:],
                            op=mybir.AluOpType.add,
                        )
                    dst_dram = x1_shard if layer == 0 else out_shard
                    # batched write: flush every WB full blocks (or tail)
                    if B == NB - 1 or (B % WB == WB - 1 and (B + 1) * P <= NP):
                        b0 = (B // WB) * WB
                        nblk = B - b0 + 1
                        r0 = b0 * P
                        r1 = min(NP, (B + 1) * P)
                        if nblk > 1 and r1 == (B + 1) * P:
                            nc.sync.dma_start(
                                dst_dram[r0:r1, :].rearrange(
                                    "(j p) d -> p j d", p=P
                                ),
                                dst_res[:, b0 * D : (B + 1) * D].rearrange(
                                    "p (j d) -> p j d", d=D
                                ),
                            )
                        else:
                            for Bj in range(b0, B + 1):
                                rj0 = Bj * P
                                rj1 = min(NP, (Bj + 1) * P)
                                nc.sync.dma_start(
                                    dst_dram[rj0:rj1, :],
                                    dst_res[: rj1 - rj0, Bj * D : (Bj + 1) * D],
                                )

                if layer == 0 and _STAGE >= 2:
                    nc.gpsimd.collective_compute(
                        "AllGather",
                        mybir.AluOpType.bypass,
                        replica_groups=[list(range(M))],
                        ins=[x1_shard.opt()],
                        outs=[x1_full.opt()],
                    )
                if layer == 0 and _STAGE < 3:
                    # debug: emit x1 as the output so the program has a writer
                    for B in range(NB):
                        rows = min(P, NP - B * P)
                        nc.sync.dma_start(
                            out_shard[B * P : B * P + rows, :],
                            x1_sb[:rows, B * D : (B + 1) * D],
                        )

    nc.compile()
    return nc


def _prepare(x, edge_index, W1_l, b1_l, W1_r, W2_l, b2_l, W2_r):
    N, _D = x.shape
    assert _D == D and N % M == 0
    NP = N // M

    src = np.asarray(edge_index[0], dtype=np.int64)
    dst = np.asarray(edge_index[1], dtype=np.int64)

    sched, idx_w, slot_w, invd_w = _build_schedule(src, dst, N, NP)
    NB = sched["NB"]

    ck = (N, NP, sched["NT"], tuple(sched["groups"]),
          tuple(t for t in sched["tile_meta"]))
    import hashlib
    hk = hashlib.sha1(repr(ck).encode()).hexdigest()
    if hk not in _prog_cache:
        _prog_cache[hk] = _build_program(sched)
    nc = _prog_cache[hk]

    x = np.ascontiguousarray(np.asarray(x, np.float32))
    xoT = np.zeros((M, D, NB * P), np.float32)
    xr = x.reshape(M, NP, D)
    xoT[:, :, :NP] = xr.transpose(0, 2, 1)

    w1l_np = np.ascontiguousarray(np.asarray(W1_l, np.float32).T)
    w1r_np = np.ascontiguousarray(np.asarray(W1_r, np.float32).T)
    w2l_np = np.ascontiguousarray(np.asarray(W2_l, np.float32).T)
    w2r_np = np.ascontiguousarray(np.asarray(W2_r, np.float32).T)
    b1_np = np.ascontiguousarray(
        np.broadcast_to(np.asarray(b1_l, np.float32), (P, D))
    )
    b2_np = np.ascontiguousarray(
        np.broadcast_to(np.asarray(b2_l, np.float32), (P, D))
    )
    iota_np = np.ascontiguousarray(
        np.tile(np.arange(P, dtype=np.float32), (P, 1))
    )
    ident_np = np.eye(P, dtype=np.float32)

    in_maps = []
    for c in range(M):
        in_maps.append({
            "x_store": x,
            "x_ownT": np.ascontiguousarray(xoT[c]),
            "idx16": idx_w[c],
            "slots": slot_w[c],
            "invd": invd_w[c],
            "w1l": w1l_np, "w1r": w1r_np, "w2l": w2l_np, "w2r": w2r_np,
            "b1": b1_np, "b2": b2_np,
            "iota": iota_np, "ident": ident_np,
        })
    return nc, in_maps


def _run(x, edge_index, W1_l, b1_l, W1_r, W2_l, b2_l, W2_r, trace=False):
    global last_bass_results
    nc, in_maps = _prepare(x, edge_index, W1_l, b1_l, W1_r, W2_l, b2_l, W2_r)
    ncores = int(os.environ.get("KERNEL_DEBUG_NCORES", str(M)))
    res = bass_utils.run_bass_kernel_spmd(
        nc, in_maps[:ncores], core_ids=list(range(ncores)), trace=trace
    )
    if ncores < M:
        out = np.concatenate(
            [res.results[c]["out_shard"] for c in range(ncores)], axis=0
        )
        last_bass_results = res
        return out
    last_bass_results = res
    out = np.concatenate([res.results[c]["out_shard"] for c in range(M)], axis=0)
    return out


def kernel(x, edge_index, W1_l, b1_l, W1_r, W2_l, b2_l, W2_r):
    return _run(x, edge_index, W1_l, b1_l, W1_r, W2_l, b2_l, W2_r, trace=False)

